# revision 13
# baseline (speedup 1.0000x reference)
"""Trainium2 Bass kernel for nn_AttentionLayer (dense transformer attention).

Full (unsharded) contract: kernel(**inputs) -> (out, attn) matching the
jax reference. Shards batch*time (B*T = 8) across the 8 NeuronCores,
one (b, t) slice per core. Self-contained: hardcodes all shapes.

Per-core math (L = S = 1024, D = 512, H = 8, DK = 64):
  q = xq @ Wq ; k = xk @ Wk ; v = xv @ Wv            (biases are zero)
  scoresT_h[s, l] = sum_e k_h[s,e] q_h[l,e]           (PE, K=64)
  maskedT = scoresT * mmT        mmT = where(mask,.1,.9).T / sqrt(DK)
  expT = exp(maskedT)                                 (ScalarE)
  [outT_h | sums_h] = [v_h | 1]^T @ expT              (PE, ones col)
  attnT_h = expT * (1/sums_h broadcast)               (DVE, bf16 2x)
  out = (vcatT * recip)^T @ Wo                        (PE, K=64 chunks)
Device emits attnT in [H, S, L] bf16; host upcasts + swapaxes view.
"""

import numpy as np
import ml_dtypes

P = 128
B, T, L, S, D, H, DK = 2, 4, 1024, 1024, 512, 8, 64
LT, ST, KC = L // P, S // P, D // P  # 8, 8, 4
NL = L // 512  # 2 free-dim chunks of 512 along l
VW = 68        # padded [v | 1] row width (4B-aligned stride for bf16)
BF16 = ml_dtypes.bfloat16

_PROGRAM_CACHE = {}


def build_program():
    """Build (once) the per-core Bass program; returns the compiled nc."""
    if "nc" in _PROGRAM_CACHE:
        return _PROGRAM_CACHE["nc"]

    import concourse.bacc as bacc
    import concourse.mybir as mybir
    import concourse.tile as tile

    f32 = mybir.dt.float32
    bf16 = mybir.dt.bfloat16

    nc = bacc.Bacc("TRN2", target_bir_lowering=False, debug=False)

    xq_d = nc.dram_tensor("xq", [L, D], bf16, kind="ExternalInput")
    xk_d = nc.dram_tensor("xk", [S, D], bf16, kind="ExternalInput")
    xv_d = nc.dram_tensor("xv", [S, D], bf16, kind="ExternalInput")
    wq_d = nc.dram_tensor("wq", [D, D], bf16, kind="ExternalInput")
    wk_d = nc.dram_tensor("wk", [D, D], bf16, kind="ExternalInput")
    wv_d = nc.dram_tensor("wv", [D, D], bf16, kind="ExternalInput")
    wo_d = nc.dram_tensor("wo", [D, D], bf16, kind="ExternalInput")
    mmt_d = nc.dram_tensor("mmt", [S, L], bf16, kind="ExternalInput")
    out_d = nc.dram_tensor("out", [L, D], f32, kind="ExternalOutput")
    # attn emitted as bf16 (host upcasts to f32; halves the dominant store)
    attnT_d = nc.dram_tensor("attnT", [H, S, L], bf16, kind="ExternalOutput")
    # internal DRAM bounce buffers for the per-head row-sum reshape
    rs1_d = nc.dram_tensor("rs_bounce1", [H, L], f32)
    rs2_d = nc.dram_tensor("rs_bounce2", [H, L], bf16)

    Exp = mybir.ActivationFunctionType.Exp

    with tile.TileContext(nc) as tc:
        with (
            tc.tile_pool(name="const", bufs=1) as const,
            tc.tile_pool(name="xT", bufs=1) as xT,
            tc.tile_pool(name="qkv", bufs=1) as qkv,
            tc.tile_pool(name="masked", bufs=4) as maskedp,
            tc.tile_pool(name="expp", bufs=2) as expp,
            tc.tile_pool(name="recipp", bufs=2) as recipp,
            tc.tile_pool(name="avsb", bufs=2) as avsbp,
            tc.tile_pool(name="attnsb", bufs=6) as attnsb,
            tc.tile_pool(name="outsb", bufs=2) as outsb,
            tc.tile_pool(name="ps_sc", bufs=3, space="PSUM") as ps_sc,
            tc.tile_pool(name="ps_av", bufs=2, space="PSUM") as ps_av,
        ):
            # ---- constant loads -------------------------------------------
            # w_sb[name][pi, k, n] = W[k*128 + pi, n]
            w_sb = {}
            for name, dram in (("wq", wq_d), ("wk", wk_d), ("wv", wv_d)):
                w = const.tile([P, KC, D], bf16, tag=f"w_{name}")
                nc.sync.dma_start(w[:], dram.rearrange("(k p) n -> p k n", p=P))
                w_sb[name] = w
            # wo in head-major 64-partition layout: wo64[pi, h, n] = Wo[h*64+pi, n]
            wo64 = const.tile([64, H, D], bf16, tag="wo64")
            nc.sync.dma_start(wo64[:], wo_d.rearrange("(h p) n -> p h n", p=64))
            # mmt[pi, st, l] = mmT[st*128 + pi, l]
            mmt = const.tile([P, ST, L], bf16, tag="mmt")
            nc.sync.dma_start(mmt[:], mmt_d.rearrange("(st p) l -> p st l", p=P))

            # ---- transposed input loads (bf16 XBAR DMA transpose) ---------
            # xT[pi, k, l] = x[l, k*128 + pi]; alternate HWDGE queues
            xqT = xT.tile([P, KC, L], bf16, tag="xqT")
            xkT = xT.tile([P, KC, S], bf16, tag="xkT")
            xvT = xT.tile([P, KC, S], bf16, tag="xvT")
            for i, (t_sb, dram) in enumerate(
                    ((xqT, xq_d), (xkT, xk_d), (xvT, xv_d))):
                for k in range(KC):
                    eng = nc.sync if (i * KC + k) % 2 == 0 else nc.scalar
                    eng.dma_start_transpose(
                        t_sb[:, k, :], dram[:, k * P:(k + 1) * P])

            # ---- projections ----------------------------------------------
            # qT[pi, m, l] = (xq @ Wq)[l, m*128 + pi]   (and same for kT)
            # chunk-major emission so head 0 (chunk 0) unblocks early
            qT = qkv.tile([P, KC, L], bf16, tag="qT")
            kT = qkv.tile([P, KC, S], bf16, tag="kT")
            for m in range(KC):
                for dst, w, xt in ((qT, w_sb["wq"], xqT), (kT, w_sb["wk"], xkT)):
                    for n in range(NL):
                        psum = ps_av.tile([P, 512], f32, tag="av")
                        for k in range(KC):
                            nc.tensor.matmul(
                                psum[:],
                                lhsT=w[:, k, m * P:(m + 1) * P],
                                rhs=xt[:, k, n * 512:(n + 1) * 512],
                                start=(k == 0), stop=(k == KC - 1))
                        nc.scalar.copy(dst[:, m, n * 512:(n + 1) * 512], psum[:])

            # v with a ones column: vs[pi, st, h, 0:64] = v[st*128+pi, h*64:..],
            # vs[..., 64] = 1.0 (row-sum trick); cols 65..67 padding, never read.
            vs = qkv.tile([P, ST, H, VW], bf16, tag="vs")
            nc.gpsimd.memset(vs[:, :, :, DK], 1.0)
            for st in range(ST):
                psum = ps_av.tile([P, 512], f32, tag="av")
                for k in range(KC):
                    nc.tensor.matmul(
                        psum[:],
                        lhsT=xvT[:, k, st * P:(st + 1) * P],
                        rhs=w_sb["wv"][:, k, :],
                        start=(k == 0), stop=(k == KC - 1))
                nc.scalar.copy(
                    vs[:, st, :, :DK],
                    psum.rearrange("p (h e) -> p h e", h=H))

            # vcat64[pi, h, l] = normalized (attn_h @ v_h)[l, pi]; partitions 0..63
            vcat64 = qkv.tile([64, H, L], bf16, tag="vcat64")

            # ---- per-head attention, software-pipelined -------------------
            # Emit scores/exp for head h BEFORE attn@V of head h-1 so the PE
            # stream stays dense (attn@V matmuls never wait on fresh exp).
            def emit_scores(h):
                po = (h % 2) * 64
                ch = h // 2
                expT = expp.tile([P, ST, L], bf16, tag="expT")
                for st in range(ST):
                    sc = ps_sc.tile([P, L], f32, tag="sc")
                    for n in range(NL):
                        nc.tensor.matmul(
                            sc[:, n * 512:(n + 1) * 512],
                            lhsT=kT[po:po + 64, ch, st * P:(st + 1) * P],
                            rhs=qT[po:po + 64, ch, n * 512:(n + 1) * 512],
                            start=True, stop=True)
                    masked = maskedp.tile([P, L], f32, tag="masked")
                    nc.vector.tensor_mul(masked[:], sc[:], mmt[:, st, :])
                    nc.scalar.activation(expT[:, st, :], masked[:], Exp)
                return expT

            def emit_attnv(h, expT):
                # attn @ [v | 1]  ->  [outT_h (rows 0..63) ; sums (row 64)]
                av = [ps_av.tile([P, 512], f32, tag="av", name=f"av{h}_{n}")
                      for n in range(NL)]
                for st in range(ST):
                    for n in range(NL):
                        nc.tensor.matmul(
                            av[n][:DK + 1, :],
                            lhsT=vs[:, st, h, :DK + 1],
                            rhs=expT[:, st, n * 512:(n + 1) * 512],
                            start=(st == 0), stop=(st == ST - 1))
                # evict unnormalized outT rows to SBUF (frees PSUM quickly)
                av_sb = avsbp.tile([64, NL, 512], bf16, tag="av_sb")
                for n in range(NL):
                    nc.scalar.copy(av_sb[:, n, :], av[n][:64, :])
                # row sums [1, 1024] -> reciprocal on 128 lanes via a DRAM
                # reshape bounce ([1,1024] -> [128,8]), then broadcast.
                # (DMA can't read PSUM, so hop the sums row through SBUF.)
                sums_sb = recipp.tile([65, L], f32, tag="sums_sb")
                for n in range(NL):
                    nsl = slice(n * 512, (n + 1) * 512)
                    nc.scalar.copy(sums_sb[64:65, nsl], av[n][64:65, :])
                    nc.sync.dma_start(rs1_d[h:h + 1, nsl], sums_sb[64:65, nsl])
                rsp = recipp.tile([P, L // P], f32, tag="rsp")
                nc.sync.dma_start(
                    rsp[:], rs1_d[h].rearrange("(p x) -> p x", p=P))
                rspr = recipp.tile([P, L // P], f32, tag="rspr")
                nc.vector.reciprocal(rspr[:], rsp[:])
                # cast f32 -> bf16 on the way out (gpsimd DMA can cast)
                nc.gpsimd.dma_start(
                    rs2_d[h].rearrange("(p x) -> p x", p=P), rspr[:])
                recip0 = recipp.tile([1, L], bf16, tag="recip0")
                nc.sync.dma_start(recip0[:1, :], rs2_d[h:h + 1, :])
                recipB = recipp.tile([P, L], bf16, tag="recipB")
                nc.gpsimd.partition_broadcast(recipB[:], recip0[:1, :])
                for n in range(NL):
                    nsl = slice(n * 512, (n + 1) * 512)
                    nc.vector.tensor_mul(
                        vcat64[:, h, nsl], av_sb[:, n, :], recipB[:64, nsl])

                # all-bf16 normalize -> DVE 2x mode; bf16 store to DRAM
                for st in range(ST):
                    att = attnsb.tile([P, L], bf16, tag="att")
                    nc.vector.tensor_mul(att[:], expT[:, st, :], recipB[:])
                    nc.sync.dma_start(attnT_d[h, st * P:(st + 1) * P, :], att[:])

            prev_expT = None
            for h in range(H):
                cur = emit_scores(h)
                if prev_expT is not None:
                    emit_attnv(h - 1, prev_expT)
                prev_expT = cur
            emit_attnv(H - 1, prev_expT)

            # ---- output projection: out = vcat @ Wo (8 chunks of K=64) ----
            for lt in range(LT):
                psum = ps_av.tile([P, 512], f32, tag="av")
                for h in range(H):
                    nc.tensor.matmul(
                        psum[:],
                        lhsT=vcat64[:, h, lt * P:(lt + 1) * P],
                        rhs=wo64[:, h, :],
                        start=(h == 0), stop=(h == H - 1))
                out_sb = outsb.tile([P, D], f32, tag="out_sb")
                nc.scalar.copy(out_sb[:], psum[:])
                nc.sync.dma_start(out_d[lt * P:(lt + 1) * P, :], out_sb[:])

    nc.compile()
    _PROGRAM_CACHE["nc"] = nc
    return nc


def make_in_maps(queries, keys, values, attn_mask, Ws):
    """Cast to bf16, build mmT, and return the 8 per-core input maps."""
    q = np.asarray(queries, dtype=np.float32).reshape(B * T, L, D)
    k = np.asarray(keys, dtype=np.float32).reshape(B * T, S, D)
    v = np.asarray(values, dtype=np.float32).reshape(B * T, S, D)
    mask = np.asarray(attn_mask)
    mmt = np.ascontiguousarray(
        (np.where(mask, np.float32(0.1), np.float32(0.9))
         * np.float32(1.0 / np.sqrt(DK))).T).astype(BF16)
    w_bf = {n: np.asarray(w, dtype=np.float32).astype(BF16)
            for n, w in Ws.items()}
    return [
        {"xq": q[bt].astype(BF16), "xk": k[bt].astype(BF16),
         "xv": v[bt].astype(BF16),
         "wq": w_bf["wq"], "wk": w_bf["wk"], "wv": w_bf["wv"],
         "wo": w_bf["wo"], "mmt": mmt}
        for bt in range(B * T)
    ]


def run_device(in_maps, trace=False, tmpdir=None):
    from concourse.bass_utils import run_bass_kernel_spmd
    nc = build_program()
    return run_bass_kernel_spmd(
        nc, in_maps, core_ids=list(range(8)), trace=trace, tmpdir=tmpdir)


def _reference_fallback(queries, keys, values, attn_mask,
                        Wq, bq, Wk, bk, Wv, bv, Wo, bo):
    """Pure-numpy fallback (only used if biases are nonzero)."""
    q = (queries @ Wq + bq).reshape(B, T, L, H, DK)
    k = (keys @ Wk + bk).reshape(B, T, S, H, DK)
    v = (values @ Wv + bv).reshape(B, T, S, H, DK)
    scores = np.einsum("btlhe,btshe->bthls", q, k)
    mm = np.where(attn_mask, np.float32(0.1), np.float32(0.9))
    scores = scores * mm * np.float32(1.0 / np.sqrt(DK))
    scores -= scores.max(axis=-1, keepdims=True)
    e = np.exp(scores)
    attn = e / e.sum(axis=-1, keepdims=True)
    V = np.einsum("bthls,btshd->btlhd", attn, v).reshape(B, T, L, H * DK)
    return (V @ Wo + bo).astype(np.float32), attn.astype(np.float32)


def kernel(queries, keys, values, attn_mask, no_tf_genes_index,
           Wq, bq, Wk, bk, Wv, bv, Wo, bo):
    queries = np.asarray(queries, dtype=np.float32)
    keys = np.asarray(keys, dtype=np.float32)
    values = np.asarray(values, dtype=np.float32)
    attn_mask = np.asarray(attn_mask)
    Ws = {"wq": Wq, "wk": Wk, "wv": Wv, "wo": Wo}
    biases = [np.asarray(b, dtype=np.float32) for b in (bq, bk, bv, bo)]
    if any(np.any(b) for b in biases):
        return _reference_fallback(
            queries, keys, values, attn_mask,
            np.asarray(Wq, np.float32), biases[0],
            np.asarray(Wk, np.float32), biases[1],
            np.asarray(Wv, np.float32), biases[2],
            np.asarray(Wo, np.float32), biases[3])

    in_maps = make_in_maps(queries, keys, values, attn_mask, Ws)
    res = run_device(in_maps)
    outs = res.results
    out_full = np.stack([r["out"] for r in outs]).reshape(B, T, L, D)
    attnT = np.stack([r["attnT"] for r in outs]).astype(np.float32)
    attn = attnT.reshape(B, T, H, S, L).swapaxes(3, 4)  # [B, T, H, L, S]
    return out_full.astype(np.float32, copy=False), attn


# revision 14
# speedup vs baseline: 1.0632x; 1.0632x over previous
"""Trainium2 Bass kernel for nn_AttentionLayer (dense transformer attention).

Full (unsharded) contract: kernel(**inputs) -> (out, attn) matching the
jax reference. Shards batch*time (B*T = 8) across the 8 NeuronCores,
one (b, t) slice per core. Self-contained: hardcodes all shapes.

Per-core math (L = S = 1024, D = 512, H = 8, DK = 64):
  q = xq @ Wq ; k = xk @ Wk ; v = xv @ Wv            (biases are zero)
  scoresT_h[s, l] = sum_e k_h[s,e] q_h[l,e]           (PE, K=64)
  maskedT = scoresT * mmT        mmT = where(mask,.1,.9).T / sqrt(DK)
  expT = exp(maskedT)                                 (ScalarE)
  [outT_h | sums_h] = [v_h | 1]^T @ expT              (PE, ones col)
  attnT_h = expT * (1/sums_h broadcast)               (DVE, bf16 2x)
  out = (vcatT * recip)^T @ Wo                        (PE, K=64 chunks)
Device emits attnT in [H, S, L] bf16; host upcasts + swapaxes view.
"""

import numpy as np
import ml_dtypes

P = 128
B, T, L, S, D, H, DK = 2, 4, 1024, 1024, 512, 8, 64
LT, ST, KC = L // P, S // P, D // P  # 8, 8, 4
NL = L // 512  # 2 free-dim chunks of 512 along l
VW = 68        # padded [v | 1] row width (4B-aligned stride for bf16)
BF16 = ml_dtypes.bfloat16

_PROGRAM_CACHE = {}


def build_program():
    """Build (once) the per-core Bass program; returns the compiled nc."""
    if "nc" in _PROGRAM_CACHE:
        return _PROGRAM_CACHE["nc"]

    import concourse.bacc as bacc
    import concourse.mybir as mybir
    import concourse.tile as tile

    f32 = mybir.dt.float32
    bf16 = mybir.dt.bfloat16

    nc = bacc.Bacc("TRN2", target_bir_lowering=False, debug=False)

    xq_d = nc.dram_tensor("xq", [L, D], bf16, kind="ExternalInput")
    xk_d = nc.dram_tensor("xk", [S, D], bf16, kind="ExternalInput")
    xv_d = nc.dram_tensor("xv", [S, D], bf16, kind="ExternalInput")
    wq_d = nc.dram_tensor("wq", [D, D], bf16, kind="ExternalInput")
    wk_d = nc.dram_tensor("wk", [D, D], bf16, kind="ExternalInput")
    wv_d = nc.dram_tensor("wv", [D, D], bf16, kind="ExternalInput")
    wo_d = nc.dram_tensor("wo", [D, D], bf16, kind="ExternalInput")
    mmt_d = nc.dram_tensor("mmt", [S, L], bf16, kind="ExternalInput")
    out_d = nc.dram_tensor("out", [L, D], f32, kind="ExternalOutput")
    # attn emitted as bf16 (host upcasts to f32; halves the dominant store)
    attnT_d = nc.dram_tensor("attnT", [H, S, L], bf16, kind="ExternalOutput")
    # internal DRAM bounce buffers for the per-head row-sum reshape
    rs1_d = nc.dram_tensor("rs_bounce1", [H, L], f32)
    rs2_d = nc.dram_tensor("rs_bounce2", [H, L], bf16)

    Exp = mybir.ActivationFunctionType.Exp

    with tile.TileContext(nc) as tc:
        with (
            tc.tile_pool(name="const", bufs=1) as const,
            tc.tile_pool(name="xT", bufs=1) as xT,
            tc.tile_pool(name="qkv", bufs=1) as qkv,
            tc.tile_pool(name="masked", bufs=4) as maskedp,
            tc.tile_pool(name="expp", bufs=2) as expp,
            tc.tile_pool(name="recipp", bufs=2) as recipp,
            tc.tile_pool(name="avsb", bufs=2) as avsbp,
            tc.tile_pool(name="attnsb", bufs=6) as attnsb,
            tc.tile_pool(name="outsb", bufs=2) as outsb,
            tc.tile_pool(name="ps_sc", bufs=3, space="PSUM") as ps_sc,
            tc.tile_pool(name="ps_av", bufs=2, space="PSUM") as ps_av,
        ):
            # ---- transposed input loads FIRST (one contiguous XBAR-
            # transpose block on one queue; transpose<->copy transitions
            # serialize the DMA XBAR, so don't interleave with plain loads)
            xqT = xT.tile([P, KC, L], bf16, tag="xqT")
            xkT = xT.tile([P, KC, S], bf16, tag="xkT")
            xvT = xT.tile([P, KC, S], bf16, tag="xvT")
            for t_sb, dram in ((xqT, xq_d), (xkT, xk_d), (xvT, xv_d)):
                for k in range(KC):
                    nc.sync.dma_start_transpose(
                        t_sb[:, k, :], dram[:, k * P:(k + 1) * P])

            # ---- constant loads (priority order: wq/wk gate the first
            # projections, mmt gates the first mask multiply) --------------
            # w_sb[name][pi, k, n] = W[k*128 + pi, n]
            w_sb = {}
            for name, dram in (("wq", wq_d), ("wk", wk_d), ("wv", wv_d)):
                w = const.tile([P, KC, D], bf16, tag=f"w_{name}")
                nc.sync.dma_start(w[:], dram.rearrange("(k p) n -> p k n", p=P))
                w_sb[name] = w
            # mmt[pi, st, l] = mmT[st*128 + pi, l]
            mmt = const.tile([P, ST, L], bf16, tag="mmt")
            nc.sync.dma_start(mmt[:], mmt_d.rearrange("(st p) l -> p st l", p=P))
            # wo in head-major 64-partition layout: wo64[pi, h, n] = Wo[h*64+pi, n]
            wo64 = const.tile([64, H, D], bf16, tag="wo64")
            nc.sync.dma_start(wo64[:], wo_d.rearrange("(h p) n -> p h n", p=64))

            # ---- projections ----------------------------------------------
            # qT[pi, m, l] = (xq @ Wq)[l, m*128 + pi]   (and same for kT)
            # chunk-major emission so head 0 (chunk 0) unblocks early
            qT = qkv.tile([P, KC, L], bf16, tag="qT")
            kT = qkv.tile([P, KC, S], bf16, tag="kT")
            for m in range(KC):
                for dst, w, xt in ((qT, w_sb["wq"], xqT), (kT, w_sb["wk"], xkT)):
                    for n in range(NL):
                        psum = ps_av.tile([P, 512], f32, tag="av")
                        for k in range(KC):
                            nc.tensor.matmul(
                                psum[:],
                                lhsT=w[:, k, m * P:(m + 1) * P],
                                rhs=xt[:, k, n * 512:(n + 1) * 512],
                                start=(k == 0), stop=(k == KC - 1))
                        nc.scalar.copy(dst[:, m, n * 512:(n + 1) * 512], psum[:])

            # v with a ones column: vs[pi, st, h, 0:64] = v[st*128+pi, h*64:..],
            # vs[..., 64] = 1.0 (row-sum trick); cols 65..67 padding, never read.
            vs = qkv.tile([P, ST, H, VW], bf16, tag="vs")
            nc.gpsimd.memset(vs[:, :, :, DK], 1.0)
            for st in range(ST):
                psum = ps_av.tile([P, 512], f32, tag="av")
                for k in range(KC):
                    nc.tensor.matmul(
                        psum[:],
                        lhsT=xvT[:, k, st * P:(st + 1) * P],
                        rhs=w_sb["wv"][:, k, :],
                        start=(k == 0), stop=(k == KC - 1))
                nc.scalar.copy(
                    vs[:, st, :, :DK],
                    psum.rearrange("p (h e) -> p h e", h=H))

            # vcat64[pi, h, l] = normalized (attn_h @ v_h)[l, pi]; partitions 0..63
            vcat64 = qkv.tile([64, H, L], bf16, tag="vcat64")

            # ---- per-head attention, software-pipelined -------------------
            # Emit scores/exp for head h BEFORE attn@V of head h-1 so the PE
            # stream stays dense (attn@V matmuls never wait on fresh exp).
            def emit_scores(h):
                po = (h % 2) * 64
                ch = h // 2
                expT = expp.tile([P, ST, L], bf16, tag="expT")
                for st in range(ST):
                    sc = ps_sc.tile([P, L], f32, tag="sc")
                    for n in range(NL):
                        nc.tensor.matmul(
                            sc[:, n * 512:(n + 1) * 512],
                            lhsT=kT[po:po + 64, ch, st * P:(st + 1) * P],
                            rhs=qT[po:po + 64, ch, n * 512:(n + 1) * 512],
                            start=True, stop=True)
                    masked = maskedp.tile([P, L], f32, tag="masked")
                    nc.vector.tensor_mul(masked[:], sc[:], mmt[:, st, :])
                    nc.scalar.activation(expT[:, st, :], masked[:], Exp)
                return expT

            def emit_attnv(h, expT):
                # attn @ [v | 1]  ->  [outT_h (rows 0..63) ; sums (row 64)]
                av = [ps_av.tile([P, 512], f32, tag="av", name=f"av{h}_{n}")
                      for n in range(NL)]
                for st in range(ST):
                    for n in range(NL):
                        nc.tensor.matmul(
                            av[n][:DK + 1, :],
                            lhsT=vs[:, st, h, :DK + 1],
                            rhs=expT[:, st, n * 512:(n + 1) * 512],
                            start=(st == 0), stop=(st == ST - 1))
                # evict unnormalized outT rows to SBUF (frees PSUM quickly)
                av_sb = avsbp.tile([64, NL, 512], bf16, tag="av_sb")
                for n in range(NL):
                    nc.scalar.copy(av_sb[:, n, :], av[n][:64, :])
                # row sums [1, 1024] -> reciprocal on 128 lanes via a DRAM
                # reshape bounce ([1,1024] -> [128,8]), then broadcast.
                # (DMA can't read PSUM, so hop the sums row through SBUF.)
                sums_sb = recipp.tile([65, L], f32, tag="sums_sb")
                for n in range(NL):
                    nsl = slice(n * 512, (n + 1) * 512)
                    nc.scalar.copy(sums_sb[64:65, nsl], av[n][64:65, :])
                    nc.sync.dma_start(rs1_d[h:h + 1, nsl], sums_sb[64:65, nsl])
                rsp = recipp.tile([P, L // P], f32, tag="rsp")
                nc.sync.dma_start(
                    rsp[:], rs1_d[h].rearrange("(p x) -> p x", p=P))
                rspr = recipp.tile([P, L // P], f32, tag="rspr")
                nc.vector.reciprocal(rspr[:], rsp[:])
                # cast f32 -> bf16 on the way out (gpsimd DMA can cast)
                nc.gpsimd.dma_start(
                    rs2_d[h].rearrange("(p x) -> p x", p=P), rspr[:])
                recip0 = recipp.tile([1, L], bf16, tag="recip0")
                nc.sync.dma_start(recip0[:1, :], rs2_d[h:h + 1, :])
                recipB = recipp.tile([P, L], bf16, tag="recipB")
                nc.gpsimd.partition_broadcast(recipB[:], recip0[:1, :])
                for n in range(NL):
                    nsl = slice(n * 512, (n + 1) * 512)
                    nc.vector.tensor_mul(
                        vcat64[:, h, nsl], av_sb[:, n, :], recipB[:64, nsl])

                # all-bf16 normalize -> DVE 2x mode; bf16 store to DRAM
                for st in range(ST):
                    att = attnsb.tile([P, L], bf16, tag="att")
                    nc.vector.tensor_mul(att[:], expT[:, st, :], recipB[:])
                    nc.sync.dma_start(attnT_d[h, st * P:(st + 1) * P, :], att[:])

            out_acc = qkv.tile([P, LT, 512], f32, tag="out_acc")

            def emit_outproj_half(h0, h1, first):
                for lt in range(LT):
                    psum = ps_av.tile([P, 512], f32, tag="av")
                    for h in range(h0, h1):
                        nc.tensor.matmul(
                            psum[:],
                            lhsT=vcat64[:, h, lt * P:(lt + 1) * P],
                            rhs=wo64[:, h, :],
                            start=(h == h0), stop=(h == h1 - 1))
                    if first:
                        nc.scalar.copy(out_acc[:, lt, :], psum[:])
                    else:
                        out_sb = outsb.tile([P, D], f32, tag="out_sb")
                        nc.vector.tensor_add(
                            out=out_sb[:], in0=psum[:], in1=out_acc[:, lt, :])
                        nc.sync.dma_start(
                            out_d[lt * P:(lt + 1) * P, :], out_sb[:])

            prev_expT = None
            for h in range(H):
                cur = emit_scores(h)
                if prev_expT is not None:
                    emit_attnv(h - 1, prev_expT)
                if h == 5:
                    emit_outproj_half(0, 4, True)
                prev_expT = cur
            emit_attnv(H - 1, prev_expT)
            emit_outproj_half(4, H, False)

    nc.compile()
    _PROGRAM_CACHE["nc"] = nc
    return nc


def make_in_maps(queries, keys, values, attn_mask, Ws):
    """Cast to bf16, build mmT, and return the 8 per-core input maps."""
    q = np.asarray(queries, dtype=np.float32).reshape(B * T, L, D)
    k = np.asarray(keys, dtype=np.float32).reshape(B * T, S, D)
    v = np.asarray(values, dtype=np.float32).reshape(B * T, S, D)
    mask = np.asarray(attn_mask)
    mmt = np.ascontiguousarray(
        (np.where(mask, np.float32(0.1), np.float32(0.9))
         * np.float32(1.0 / np.sqrt(DK))).T).astype(BF16)
    w_bf = {n: np.asarray(w, dtype=np.float32).astype(BF16)
            for n, w in Ws.items()}
    return [
        {"xq": q[bt].astype(BF16), "xk": k[bt].astype(BF16),
         "xv": v[bt].astype(BF16),
         "wq": w_bf["wq"], "wk": w_bf["wk"], "wv": w_bf["wv"],
         "wo": w_bf["wo"], "mmt": mmt}
        for bt in range(B * T)
    ]


def run_device(in_maps, trace=False, tmpdir=None):
    from concourse.bass_utils import run_bass_kernel_spmd
    nc = build_program()
    return run_bass_kernel_spmd(
        nc, in_maps, core_ids=list(range(8)), trace=trace, tmpdir=tmpdir)


def _reference_fallback(queries, keys, values, attn_mask,
                        Wq, bq, Wk, bk, Wv, bv, Wo, bo):
    """Pure-numpy fallback (only used if biases are nonzero)."""
    q = (queries @ Wq + bq).reshape(B, T, L, H, DK)
    k = (keys @ Wk + bk).reshape(B, T, S, H, DK)
    v = (values @ Wv + bv).reshape(B, T, S, H, DK)
    scores = np.einsum("btlhe,btshe->bthls", q, k)
    mm = np.where(attn_mask, np.float32(0.1), np.float32(0.9))
    scores = scores * mm * np.float32(1.0 / np.sqrt(DK))
    scores -= scores.max(axis=-1, keepdims=True)
    e = np.exp(scores)
    attn = e / e.sum(axis=-1, keepdims=True)
    V = np.einsum("bthls,btshd->btlhd", attn, v).reshape(B, T, L, H * DK)
    return (V @ Wo + bo).astype(np.float32), attn.astype(np.float32)


def kernel(queries, keys, values, attn_mask, no_tf_genes_index,
           Wq, bq, Wk, bk, Wv, bv, Wo, bo):
    queries = np.asarray(queries, dtype=np.float32)
    keys = np.asarray(keys, dtype=np.float32)
    values = np.asarray(values, dtype=np.float32)
    attn_mask = np.asarray(attn_mask)
    Ws = {"wq": Wq, "wk": Wk, "wv": Wv, "wo": Wo}
    biases = [np.asarray(b, dtype=np.float32) for b in (bq, bk, bv, bo)]
    if any(np.any(b) for b in biases):
        return _reference_fallback(
            queries, keys, values, attn_mask,
            np.asarray(Wq, np.float32), biases[0],
            np.asarray(Wk, np.float32), biases[1],
            np.asarray(Wv, np.float32), biases[2],
            np.asarray(Wo, np.float32), biases[3])

    in_maps = make_in_maps(queries, keys, values, attn_mask, Ws)
    res = run_device(in_maps)
    outs = res.results
    out_full = np.stack([r["out"] for r in outs]).reshape(B, T, L, D)
    attnT = np.stack([r["attnT"] for r in outs]).astype(np.float32)
    attn = attnT.reshape(B, T, H, S, L).swapaxes(3, 4)  # [B, T, H, L, S]
    return out_full.astype(np.float32, copy=False), attn


# revision 16
# speedup vs baseline: 1.2409x; 1.1672x over previous
"""Trainium2 Bass kernel for nn_AttentionLayer (dense transformer attention).

Full (unsharded) contract: kernel(**inputs) -> (out, attn) matching the
jax reference. Shards batch*time (B*T = 8) across the 8 NeuronCores,
one (b, t) slice per core. Self-contained: hardcodes all shapes.

Per-core math (L = S = 1024, D = 512, H = 8, DK = 64):
  q = xq @ Wq ; k = xk @ Wk ; v = xv @ Wv            (biases are zero)
  scoresT_h[s, l] = sum_e k_h[s,e] q_h[l,e]           (PE, K=64)
  maskedT = scoresT * mmT        mmT = where(mask,.1,.9).T / sqrt(DK)
  expT = exp(maskedT)                                 (ScalarE)
  [outT_h | sums_h] = [v_h | 1]^T @ expT              (PE, ones col)
  attnT_h = expT * (1/sums_h broadcast)               (DVE, bf16 2x)
  out = (vcatT * recip)^T @ Wo                        (PE, K=64 chunks)

Two-deep software pipeline: block h emits scores/exp of head h,
attn@V of head h-1, and normalize/stores of head h-2, interleaved at
s-tile granularity so the PE and DVE streams never stall on the
row-sum reciprocal chain. Device emits attnT in [H, S, L] bf16; host
upcasts + returns a swapaxes view. Inputs arrive pre-transposed from
the host (xqT/xkT/xvT = x.T) so no on-device transposes are needed.
"""

import numpy as np
import ml_dtypes

P = 128
B, T, L, S, D, H, DK = 2, 4, 1024, 1024, 512, 8, 64
LT, ST, KC = L // P, S // P, D // P  # 8, 8, 4
NL = L // 512  # 2 free-dim chunks of 512 along l
VW = 68        # padded [v | 1] row width (4B-aligned stride for bf16)
BF16 = ml_dtypes.bfloat16

_PROGRAM_CACHE = {}


def build_program():
    """Build (once) the per-core Bass program; returns the compiled nc."""
    if "nc" in _PROGRAM_CACHE:
        return _PROGRAM_CACHE["nc"]

    import concourse.bacc as bacc
    import concourse.mybir as mybir
    import concourse.tile as tile

    f32 = mybir.dt.float32
    bf16 = mybir.dt.bfloat16

    nc = bacc.Bacc("TRN2", target_bir_lowering=False, debug=False)

    xqT_d = nc.dram_tensor("xqT", [D, L], bf16, kind="ExternalInput")
    xkT_d = nc.dram_tensor("xkT", [D, S], bf16, kind="ExternalInput")
    xvT_d = nc.dram_tensor("xvT", [D, S], bf16, kind="ExternalInput")
    wq_d = nc.dram_tensor("wq", [D, D], bf16, kind="ExternalInput")
    wk_d = nc.dram_tensor("wk", [D, D], bf16, kind="ExternalInput")
    wv_d = nc.dram_tensor("wv", [D, D], bf16, kind="ExternalInput")
    wo_d = nc.dram_tensor("wo", [D, D], bf16, kind="ExternalInput")
    mmt_d = nc.dram_tensor("mmt", [S, L], bf16, kind="ExternalInput")
    out_d = nc.dram_tensor("out", [L, D], f32, kind="ExternalOutput")
    # attn emitted as bf16 (host upcasts to f32; halves the dominant store)
    attnT_d = nc.dram_tensor("attnT", [H, S, L], bf16, kind="ExternalOutput")
    # internal DRAM bounce buffers for the per-head row-sum reshape
    rs1_d = nc.dram_tensor("rs_bounce1", [H, L], f32)
    rs2_d = nc.dram_tensor("rs_bounce2", [H, L], bf16)

    Exp = mybir.ActivationFunctionType.Exp

    with tile.TileContext(nc) as tc:
        with (
            tc.tile_pool(name="const", bufs=1) as const,
            tc.tile_pool(name="xT", bufs=1) as xT,
            tc.tile_pool(name="qkv", bufs=1) as qkv,
            tc.tile_pool(name="masked", bufs=3) as maskedp,
            tc.tile_pool(name="expp", bufs=3) as expp,
            tc.tile_pool(name="recipp", bufs=2) as recipp,
            tc.tile_pool(name="avsb", bufs=2) as avsbp,
            tc.tile_pool(name="attnsb", bufs=4) as attnsb,
            tc.tile_pool(name="outsb", bufs=2) as outsb,
            tc.tile_pool(name="ps_sc", bufs=2, space="PSUM") as ps_sc,
            tc.tile_pool(name="ps_av", bufs=2, space="PSUM") as ps_av,
            tc.tile_pool(name="ps_op", bufs=2, space="PSUM") as ps_op,
        ):
            # ---- loads, split across the two HWDGE queues -----------------
            # sync: wq, xqT, mmt, wv;  scalar: wk, xkT, xvT, wo
            # (wq/wk + xqT/xkT gate the first projections)
            w_sb = {}
            for name in ("wq", "wk", "wv"):
                w_sb[name] = const.tile([P, KC, D], bf16, tag=f"w_{name}",
                                        name=f"w_{name}")
            wo64 = const.tile([64, H, D], bf16, tag="wo64")
            mmt = const.tile([P, ST, L], bf16, tag="mmt")
            xqT = xT.tile([P, KC, L], bf16, tag="xqT")
            xkT = xT.tile([P, KC, S], bf16, tag="xkT")
            xvT = xT.tile([P, KC, S], bf16, tag="xvT")

            nc.sync.dma_start(w_sb["wq"][:],
                              wq_d.rearrange("(k p) n -> p k n", p=P))
            nc.scalar.dma_start(w_sb["wk"][:],
                                wk_d.rearrange("(k p) n -> p k n", p=P))
            nc.sync.dma_start(xqT[:], xqT_d.rearrange("(k p) l -> p k l", p=P))
            nc.scalar.dma_start(xkT[:], xkT_d.rearrange("(k p) l -> p k l", p=P))
            nc.sync.dma_start(mmt[:], mmt_d.rearrange("(st p) l -> p st l", p=P))
            nc.scalar.dma_start(xvT[:], xvT_d.rearrange("(k p) l -> p k l", p=P))
            nc.sync.dma_start(w_sb["wv"][:],
                              wv_d.rearrange("(k p) n -> p k n", p=P))
            nc.scalar.dma_start(wo64[:], wo_d.rearrange("(h p) n -> p h n", p=64))

            # ---- projections (chunk-major: head 0 needs only chunk 0) -----
            # qT[pi, m, l] = (xq @ Wq)[l, m*128 + pi]   (and same for kT)
            qT = qkv.tile([P, KC, L], bf16, tag="qT")
            kT = qkv.tile([P, KC, S], bf16, tag="kT")
            for m in range(KC):
                for dst, w, xt in ((qT, w_sb["wq"], xqT), (kT, w_sb["wk"], xkT)):
                    for n in range(NL):
                        psum = ps_op.tile([P, 512], f32, tag="op")
                        for k in range(KC):
                            nc.tensor.matmul(
                                psum[:],
                                lhsT=w[:, k, m * P:(m + 1) * P],
                                rhs=xt[:, k, n * 512:(n + 1) * 512],
                                start=(k == 0), stop=(k == KC - 1))
                        nc.scalar.copy(dst[:, m, n * 512:(n + 1) * 512], psum[:])

            # v with a ones column: vs[pi, st, h, 0:64] = v[st*128+pi, h*64:..],
            # vs[..., 64] = 1.0 (row-sum trick); cols 65..67 padding, never read.
            vs = qkv.tile([P, ST, H, VW], bf16, tag="vs")
            nc.gpsimd.memset(vs[:, :, :, DK], 1.0)
            for st in range(ST):
                psum = ps_op.tile([P, 512], f32, tag="op")
                for k in range(KC):
                    nc.tensor.matmul(
                        psum[:],
                        lhsT=xvT[:, k, st * P:(st + 1) * P],
                        rhs=w_sb["wv"][:, k, :],
                        start=(k == 0), stop=(k == KC - 1))
                nc.scalar.copy(
                    vs[:, st, :, :DK],
                    psum.rearrange("p (h e) -> p h e", h=H))

            # vcat64[pi, h, l] = normalized (attn_h @ v_h)[l, pi]; partitions 0..63
            vcat64 = qkv.tile([64, H, L], bf16, tag="vcat64")
            out_acc = qkv.tile([P, LT, 512], f32, tag="out_acc")

            # ---- per-head attention: two-deep software pipeline -----------
            state = {}  # h -> dict(expT=, av=, av_sb=, recipB=)

            def emit_sums_chain(h):
                """av sums row -> reciprocal on 128 lanes (DRAM reshape
                bounce) -> bf16 -> DMA partition-broadcast to recipB."""
                s = state[h]
                av = s["av"]
                av_sb = avsbp.tile([64, NL, 512], bf16, tag="av_sb")
                sums_sb = recipp.tile([65, L], f32, tag="sums_sb")
                for n in range(NL):
                    nsl = slice(n * 512, (n + 1) * 512)
                    nc.scalar.copy(av_sb[:, n, :], av[n][:64, :])
                    nc.scalar.copy(sums_sb[64:65, nsl], av[n][64:65, :])
                    nc.sync.dma_start(rs1_d[h:h + 1, nsl], sums_sb[64:65, nsl])
                rsp = recipp.tile([P, L // P], f32, tag="rsp")
                nc.sync.dma_start(
                    rsp[:], rs1_d[h].rearrange("(p x) -> p x", p=P))
                rspr = recipp.tile([P, L // P], f32, tag="rspr")
                nc.vector.reciprocal(rspr[:], rsp[:])
                # cast f32 -> bf16 on the way out (gpsimd DMA can cast)
                nc.gpsimd.dma_start(
                    rs2_d[h].rearrange("(p x) -> p x", p=P), rspr[:])
                recipB = recipp.tile([P, L], bf16, tag="recipB")
                nc.sync.dma_start(
                    recipB[:], rs2_d[h:h + 1, :].to_broadcast((P, L)))
                s["av_sb"] = av_sb
                s["recipB"] = recipB

            def emit_block(h):
                """h: scores+mask+exp; h-1: attn@V; h-2: normalize+stores."""
                do_sc = h < H
                do_av = 1 <= h <= H
                do_nm = 2 <= h
                if do_sc:
                    po = (h % 2) * 64
                    ch = h // 2
                    expT = expp.tile([P, ST, L], bf16, tag="expT")
                    state[h] = {"expT": expT}
                if do_av:
                    sa = state[h - 1]
                    sa["av"] = [
                        ps_av.tile([P, 512], f32, tag="av", name=f"av{h - 1}_{n}")
                        for n in range(NL)]
                if do_nm:
                    sn = state[h - 2]

                for st in range(ST):
                    if do_sc:
                        sc = ps_sc.tile([P, L], f32, tag="sc")
                        for n in range(NL):
                            nc.tensor.matmul(
                                sc[:, n * 512:(n + 1) * 512],
                                lhsT=kT[po:po + 64, ch, st * P:(st + 1) * P],
                                rhs=qT[po:po + 64, ch, n * 512:(n + 1) * 512],
                                start=True, stop=True)
                    if do_av:
                        for n in range(NL):
                            nc.tensor.matmul(
                                sa["av"][n][:DK + 1, :],
                                lhsT=vs[:, st, h - 1, :DK + 1],
                                rhs=sa["expT"][:, st, n * 512:(n + 1) * 512],
                                start=(st == 0), stop=(st == ST - 1))
                    if do_sc:
                        masked = maskedp.tile([P, L], f32, tag="masked")
                        nc.vector.tensor_mul(masked[:], sc[:], mmt[:, st, :])
                        nc.scalar.activation(expT[:, st, :], masked[:], Exp)
                    if do_nm:
                        att = attnsb.tile([P, L], bf16, tag="att")
                        nc.vector.tensor_mul(
                            att[:], sn["expT"][:, st, :], sn["recipB"][:])
                        nc.sync.dma_start(
                            attnT_d[h - 2, st * P:(st + 1) * P, :], att[:])

                if do_av:
                    emit_sums_chain(h - 1)
                if do_nm:
                    # vcat (normalized attn@V rows) for head h-2
                    for n in range(NL):
                        nsl = slice(n * 512, (n + 1) * 512)
                        nc.vector.tensor_mul(
                            vcat64[:, h - 2, nsl],
                            sn["av_sb"][:, n, :], sn["recipB"][:64, nsl])

            def emit_outproj_half(h0, h1, first):
                for lt in range(LT):
                    psum = ps_op.tile([P, 512], f32, tag="op")
                    for h in range(h0, h1):
                        nc.tensor.matmul(
                            psum[:],
                            lhsT=vcat64[:, h, lt * P:(lt + 1) * P],
                            rhs=wo64[:, h, :],
                            start=(h == h0), stop=(h == h1 - 1))
                    if first:
                        nc.scalar.copy(out_acc[:, lt, :], psum[:])
                    else:
                        out_sb = outsb.tile([P, D], f32, tag="out_sb")
                        nc.vector.tensor_add(
                            out=out_sb[:], in0=psum[:], in1=out_acc[:, lt, :])
                        nc.sync.dma_start(
                            out_d[lt * P:(lt + 1) * P, :], out_sb[:])

            for h in range(H + 2):
                emit_block(h)
                if h == 5:  # vcat64[0..3] complete after block 5
                    emit_outproj_half(0, 4, True)
            emit_outproj_half(4, H, False)

    nc.compile()
    _PROGRAM_CACHE["nc"] = nc
    return nc


def make_in_maps(queries, keys, values, attn_mask, Ws):
    """Cast to bf16 (pre-transposing x), build mmT, return per-core maps."""
    q = np.asarray(queries, dtype=np.float32).reshape(B * T, L, D)
    k = np.asarray(keys, dtype=np.float32).reshape(B * T, S, D)
    v = np.asarray(values, dtype=np.float32).reshape(B * T, S, D)
    mask = np.asarray(attn_mask)
    mmt = np.ascontiguousarray(
        (np.where(mask, np.float32(0.1), np.float32(0.9))
         * np.float32(1.0 / np.sqrt(DK))).T).astype(BF16)
    w_bf = {n: np.asarray(w, dtype=np.float32).astype(BF16)
            for n, w in Ws.items()}
    return [
        {"xqT": q[bt].T.astype(BF16), "xkT": k[bt].T.astype(BF16),
         "xvT": v[bt].T.astype(BF16),
         "wq": w_bf["wq"], "wk": w_bf["wk"], "wv": w_bf["wv"],
         "wo": w_bf["wo"], "mmt": mmt}
        for bt in range(B * T)
    ]


def run_device(in_maps, trace=False, tmpdir=None):
    from concourse.bass_utils import run_bass_kernel_spmd
    nc = build_program()
    return run_bass_kernel_spmd(
        nc, in_maps, core_ids=list(range(8)), trace=trace, tmpdir=tmpdir)


def _reference_fallback(queries, keys, values, attn_mask,
                        Wq, bq, Wk, bk, Wv, bv, Wo, bo):
    """Pure-numpy fallback (only used if biases are nonzero)."""
    q = (queries @ Wq + bq).reshape(B, T, L, H, DK)
    k = (keys @ Wk + bk).reshape(B, T, S, H, DK)
    v = (values @ Wv + bv).reshape(B, T, S, H, DK)
    scores = np.einsum("btlhe,btshe->bthls", q, k)
    mm = np.where(attn_mask, np.float32(0.1), np.float32(0.9))
    scores = scores * mm * np.float32(1.0 / np.sqrt(DK))
    scores -= scores.max(axis=-1, keepdims=True)
    e = np.exp(scores)
    attn = e / e.sum(axis=-1, keepdims=True)
    V = np.einsum("bthls,btshd->btlhd", attn, v).reshape(B, T, L, H * DK)
    return (V @ Wo + bo).astype(np.float32), attn.astype(np.float32)


def kernel(queries, keys, values, attn_mask, no_tf_genes_index,
           Wq, bq, Wk, bk, Wv, bv, Wo, bo):
    queries = np.asarray(queries, dtype=np.float32)
    keys = np.asarray(keys, dtype=np.float32)
    values = np.asarray(values, dtype=np.float32)
    attn_mask = np.asarray(attn_mask)
    Ws = {"wq": Wq, "wk": Wk, "wv": Wv, "wo": Wo}
    biases = [np.asarray(b, dtype=np.float32) for b in (bq, bk, bv, bo)]
    if any(np.any(b) for b in biases):
        return _reference_fallback(
            queries, keys, values, attn_mask,
            np.asarray(Wq, np.float32), biases[0],
            np.asarray(Wk, np.float32), biases[1],
            np.asarray(Wv, np.float32), biases[2],
            np.asarray(Wo, np.float32), biases[3])

    in_maps = make_in_maps(queries, keys, values, attn_mask, Ws)
    res = run_device(in_maps)
    outs = res.results
    out_full = np.stack([r["out"] for r in outs]).reshape(B, T, L, D)
    attnT = np.stack([r["attnT"] for r in outs]).astype(np.float32)
    attn = attnT.reshape(B, T, H, S, L).swapaxes(3, 4)  # [B, T, H, L, S]
    return out_full.astype(np.float32, copy=False), attn
